# revision 3
# baseline (speedup 1.0000x reference)
"""Single-head attention kernel for TRN2, 8 NeuronCores — bf16 rewrite.

Problem: hidden [4,4096,1024] fp32; Wq/Wk/Wv [1024,64]; out [4,4096,64]
  q,k,v = hidden @ W + b ; out = softmax(q k^T / 8) @ v

Sharding: 2 cores per batch; each core computes 2048 query rows against the
full 4096-key sequence (host rotates hidden so own q-rows are rows 0:2048;
softmax over keys is permutation invariant).

v2 design (vs f32r baseline):
- hidden/W cast to bf16 on HOST: halves the 16 MiB/core hidT DMA (the
  DMA_ENGINES device is a 360 GB/s shared resource) and all matmuls run
  1 cyc/row at bf16 anyway. End-to-end rel err ~5e-3 (gate 2e-2).
- AV uses exp-weights as the STATIONARY operand: out[q=128, v|1 = 65] per
  (q-subtile, k-tile) costs only 65 rows vs 512 in the [65, 512] layout,
  and the output lands directly in [q, h] order — no epilogue transposes.
  PSUM zero-regions (2KB) allow only a few live accumulation groups, so
  keys are swept in 3 segments {4,14,14} with SBUF accumulator spills.
- exp: exact table exp on ACT (the bottleneck: 8.4M exps = 54.6us floor);
  a tunable subset of key-tile pairs is offloaded to DVE using an integer
  bf16-bit construction: z=int16(s*16*log2e+16256) then a parabolic
  mantissa correction z + g*(m^2-128m), m=z&127 — max weight err 0.9%
  (plain Schraudolph's 3% sawtooth fails the concentrated-softmax rows).
- k-tile segments sized {4,14,14} so the first exp fires ~4us in (chunk-0
  K/V only) while later segments ride the DMA stream.
"""

import numpy as np

E, S, H = 1024, 4096, 64
NT = E // 128  # 8 e-tiles
SQ = S // 2  # 2048 query rows per core
NK = S // 128  # 32 k-tiles
N_CORES = 8
SEGS = [(0, 8), (8, 20), (20, 32)]
# (seg, chunk, pair-in-seg) triples whose exp runs on DVE (int bf16 path).
# Only pair 0 of a block: the psum read happens at block start (score buffer
# freed immediately) and the int16 chain has the whole block to finish
# before the AV of this block is emitted one block later.
OFF = {(s, ci, 0) for s in (1, 2) for ci in range(4)}

_NC = None
LAST_RESULT = None


def _build():
    from contextlib import ExitStack
    import concourse.tile as tile
    from concourse import bacc, mybir
    from concourse.masks import make_identity

    F32 = mybir.dt.float32
    BF16 = mybir.dt.bfloat16
    I16 = mybir.dt.int16
    Exp = mybir.ActivationFunctionType.Exp
    Ident = mybir.ActivationFunctionType.Identity
    MULT = mybir.AluOpType.mult
    ADD = mybir.AluOpType.add
    SUB = mybir.AluOpType.subtract
    AND = mybir.AluOpType.bitwise_and

    # DVE exp constants: bf16 bits = s*(128*log2e/8) + 127*128, then
    # parabola correction gamma*(m-128)*m, m = bits & 127
    EXP_A = float(128 * np.log2(np.e) * 0.125)
    EXP_B = 16256.0
    EXP_G = 0.00255

    nc = bacc.Bacc("TRN2", target_bir_lowering=False, debug=False)
    HIDT = nc.dram_tensor("hidT", [E, S], BF16, kind="ExternalInput")
    WQ = nc.dram_tensor("wq", [E, H], BF16, kind="ExternalInput")
    WKV = nc.dram_tensor("wkv", [E, 2 * H], BF16, kind="ExternalInput")
    BQ = nc.dram_tensor("bq", [H, 1], F32, kind="ExternalInput")
    BKV = nc.dram_tensor("bkv", [2 * H, 1], F32, kind="ExternalInput")
    OUT = nc.dram_tensor("out", [SQ, H], F32, kind="ExternalOutput")

    with tile.TileContext(nc) as tc, ExitStack() as ctx:
        consts = ctx.enter_context(tc.tile_pool(name="consts", bufs=1))
        hidp = ctx.enter_context(tc.tile_pool(name="hid", bufs=1))
        stage = ctx.enter_context(tc.tile_pool(name="stage", bufs=2))
        wtp = ctx.enter_context(tc.tile_pool(name="wt", bufs=3))
        zp = ctx.enter_context(tc.tile_pool(name="zp", bufs=2))
        pps = ctx.enter_context(tc.tile_pool(name="pps", bufs=2, space="PSUM"))
        scp = ctx.enter_context(tc.tile_pool(name="scp", bufs=2, space="PSUM"))
        avp = ctx.enter_context(tc.tile_pool(name="avp", bufs=2, space="PSUM"))

        # ---- constants ----
        # identity FIRST: make_identity runs on the gpsimd queue and must not
        # sit behind the const DMAs (the v-transposes of chunk 0 need it)
        identf = consts.tile([128, 128], F32)
        make_identity(nc, identf[:])
        identb = consts.tile([128, 128], BF16)
        nc.vector.tensor_copy(identb[:], identf[:])

        wq_sb = consts.tile([128, NT, H], BF16)
        nc.gpsimd.dma_start(wq_sb[:], WQ[:].rearrange("(t p) c -> p t c", p=128))
        wkv_sb = consts.tile([128, NT, 2 * H], BF16)
        nc.gpsimd.dma_start(wkv_sb[:], WKV[:].rearrange("(t p) c -> p t c", p=128))
        bq_sb = consts.tile([H, 1], F32)
        nc.gpsimd.dma_start(bq_sb[:], BQ[:])
        bkv_sb = consts.tile([2 * H, 1], F32)
        nc.gpsimd.dma_start(bkv_sb[:], BKV[:])

        kT = consts.tile([64, S], BF16)
        qT = consts.tile([64, SQ], BF16)
        vones = consts.tile([128, NK, H + 1], BF16)
        nc.vector.memset(vones[:, :, H : H + 1], 1.0)
        acc0 = consts.tile([128, 16, H + 1], F32)
        acc1 = consts.tile([128, 16, H + 1], F32)
        res = consts.tile([128, 16, H], F32)
        hidT_sb = hidp.tile([128, NT, S], BF16)

        # warm the Exp table so the first real exp doesn't pay the load
        warm = consts.tile([1, 1], F32)
        nc.vector.memset(warm[:], 0.0)
        nc.scalar.activation(warm[:], warm[:], Exp)

        # ---- all hidT DMAs upfront. Chunk 0 fine-grained ([128,512]) so the
        # first projection can chase individual pieces; the rest as
        # [128,1024] double-chunks — each DMA instruction holds the shared
        # HWDGE generator 625ns, so fewer/bigger transfers keep the stream
        # at the 360GB/s DMA-engine bandwidth instead of HWDGE-paced.
        def dma_hid(lo, hi, t):
            nc.sync.dma_start(
                hidT_sb[:, t, lo:hi], HIDT[128 * t : 128 * (t + 1), lo:hi]
            )

        for t in range(NT):
            dma_hid(0, 512, t)
        for t in range(NT):
            dma_hid(512, 1024, t)
        for cp in ((1024, 2048), (2048, 3072), (3072, 4096)):
            for t in range(NT):
                dma_hid(cp[0], cp[1], t)

        def pe_warm(n, cols=128):
            # junk transposes: keep the PE busy through DMA-wait gaps so the
            # p-state ramp (3us continuous -> 2.4GHz) doesn't reset; early
            # projection matmuls otherwise run at the 0.65/1.2GHz p-states
            for _ in range(n):
                junk = avp.tile([128, 512], F32, tag="av")
                nc.tensor.transpose(
                    junk[0:cols, 0:cols],
                    identf[0:cols, 0:cols],
                    identf[0:cols, 0:cols],
                )

        def q_chunk(c, on_act=False, warm=0):
            pq = pps.tile([64, 512], F32, tag="pp")
            for t in range(NT):
                if warm:
                    pe_warm(warm, cols=64)
                nc.tensor.matmul(
                    pq[:],
                    wq_sb[:, t, :],
                    hidT_sb[:, t, 512 * c : 512 * (c + 1)],
                    start=(t == 0),
                    stop=(t == NT - 1),
                )
            dst = qT[:, 512 * c : 512 * (c + 1)]
            if on_act:
                nc.scalar.activation(dst, pq[:], Ident, bias=bq_sb[:])
            else:
                nc.vector.tensor_scalar_add(dst, pq[:], bq_sb[:])

        def kv_chunk(c, on_act=False, warm=0):
            pkv = pps.tile([128, 512], F32, tag="pp")
            for t in range(NT):
                if warm:
                    pe_warm(warm)
                nc.tensor.matmul(
                    pkv[:],
                    wkv_sb[:, t, :],
                    hidT_sb[:, t, 512 * c : 512 * (c + 1)],
                    start=(t == 0),
                    stop=(t == NT - 1),
                )
            kdst = kT[:, 512 * c : 512 * (c + 1)]
            if on_act:
                nc.scalar.activation(kdst, pkv[0:64, :], Ident, bias=bkv_sb[0:64, :])
            else:
                nc.vector.tensor_scalar_add(kdst, pkv[0:64, :], bkv_sb[0:64, :])
            vstg = stage.tile([64, 512], BF16, tag="vstg")
            nc.vector.tensor_scalar_add(vstg[:], pkv[64:128, :], bkv_sb[64:128, :])
            pv = pps.tile([128, 4, 64], BF16, tag="pp")
            for j in range(4):
                nc.tensor.transpose(
                    pv[:, j, :], vstg[:, 128 * j : 128 * (j + 1)], identb[0:64, 0:64]
                )
            nc.vector.tensor_copy(vones[:, 4 * c : 4 * c + 4, 0:64], pv[:])

        # ---- emission ----
        pe_warm(8)
        q_chunk(0, on_act=True, warm=4)
        kv_chunk(0, on_act=True)
        kv_chunk(1)  # segment 0 spans kt 0..7 = key chunks 0-1

        # light q-projections lead each pop batch so qT for block (0,c+1)
        # isn't queued behind an 8-matmul KV projection on the in-order PE
        pieces = [
            lambda: q_chunk(1),
            lambda: q_chunk(2),
            lambda: kv_chunk(2),
            lambda: q_chunk(3),
            lambda: kv_chunk(3),
            lambda: kv_chunk(4),
            lambda: kv_chunk(5),
            lambda: kv_chunk(6),
            lambda: kv_chunk(7),
        ]
        # pieces popped after each (seg, chunk) attention block
        POPS = {(0, 0): 2, (0, 1): 1, (0, 2): 2, (0, 3): 1, (1, 0): 1,
                (1, 1): 1, (1, 2): 1}

        def av_block(s, ci, wt, k0, k1):
            for j in range(4):
                av = avp.tile([128, 512], F32, tag="av")
                for kt in range(k0, k1):
                    nc.tensor.matmul(
                        av[:, 0 : H + 1],
                        wt[:, kt - k0, 128 * j : 128 * (j + 1)],
                        vones[:, kt, :],
                        start=(kt == k0),
                        stop=(kt == k1 - 1),
                    )
                J = 4 * ci + j
                if s == 0:
                    nc.vector.tensor_copy(acc0[:, J, :], av[:, 0 : H + 1])
                elif s == 1:
                    nc.vector.scalar_tensor_tensor(
                        acc1[:, J, :], av[:, 0 : H + 1], 1.0, acc0[:, J, :],
                        MULT, ADD,
                    )
                else:
                    fin = stage.tile([128, H + 1], F32, tag="fin")
                    nc.vector.scalar_tensor_tensor(
                        fin[:], av[:, 0 : H + 1], 1.0, acc1[:, J, :], MULT, ADD
                    )
                    rec = stage.tile([128, 1], F32, tag="rec")
                    nc.vector.reciprocal(rec[:], fin[:, H : H + 1])
                    nc.vector.tensor_scalar_mul(res[:, J, :], fin[:, 0:H], rec[:])
                    if ci == 3:
                        # final chunk: per-subtile DMA so j0-j2 stores overlap
                        # the remaining epilogue instead of one DMA at the end
                        nc.sync.dma_start(
                            OUT[128 * J : 128 * (J + 1), :], res[:, J, :]
                        )
                    elif j == 3:
                        nc.sync.dma_start(
                            OUT[512 * ci : 512 * (ci + 1), :].rearrange(
                                "(jj p) c -> p jj c", p=128
                            ),
                            res[:, 4 * ci : 4 * ci + 4, :],
                        )

        # software pipeline: emit block N's scores+exp, then block N-1's AV,
        # so ACT never waits on AV/projection matmuls at block boundaries
        def score_exp_pair(s, ci, i, k0, wt):
            qs = slice(512 * ci, 512 * (ci + 1))
            sc = scp.tile([128, 2, 512], F32, tag="sc")
            for hf in range(2):
                kt = k0 + 2 * i + hf
                nc.tensor.matmul(
                    sc[:, hf, :],
                    kT[:, 128 * kt : 128 * (kt + 1)],
                    qT[:, qs],
                    start=True,
                    stop=True,
                )
            wslice = wt[:, 2 * i : 2 * i + 2, :]
            if (s, ci, i) in OFF:
                # integer bf16-bit exp on DVE; every op after the psum
                # read is int16-only so it runs in the 2x DVE mode
                z = zp.tile([128, 2, 512], I16, tag="z")
                nc.vector.tensor_scalar(z[:], sc[:], EXP_A, EXP_B, MULT, ADD)
                m = zp.tile([128, 2, 512], I16, tag="m")
                nc.vector.tensor_scalar(m[:], z[:], 127, None, AND)
                ms = zp.tile([128, 2, 512], I16, tag="ms")
                nc.vector.tensor_scalar(ms[:], m[:], 128, None, SUB)
                nc.vector.tensor_tensor(m[:], ms[:], m[:], MULT)
                nc.vector.tensor_scalar(m[:], m[:], EXP_G, None, MULT)
                nc.vector.tensor_tensor(wslice.bitcast(I16), z[:], m[:], ADD)
            else:
                nc.scalar.activation(wslice, sc[:], Exp, scale=0.125)

        prev = None
        for s, (k0, k1) in enumerate(SEGS):
            nkt = k1 - k0
            for ci in range(4):
                wt = wtp.tile([128, 14, 512], BF16, tag="wt")
                for i in range(nkt // 2):
                    score_exp_pair(s, ci, i, k0, wt)
                if prev is not None:
                    av_block(*prev)
                for _ in range(POPS.get((s, ci), 0)):
                    pieces.pop(0)()
                prev = (s, ci, wt, k0, k1)
        av_block(*prev)
        assert not pieces, f"{len(pieces)} pieces never emitted"

    nc.compile()
    return nc


def kernel(hidden_states, Wq, bq, Wk, bk, Wv, bv):
    global _NC, LAST_RESULT
    import ml_dtypes
    from concourse.bass_utils import run_bass_kernel_spmd

    BF = ml_dtypes.bfloat16
    hidden_states = np.asarray(hidden_states, dtype=np.float32)
    assert hidden_states.shape == (4, S, E), hidden_states.shape

    if _NC is None:
        _NC = _build()

    wq = np.asarray(Wq, np.float32).astype(BF)
    wkv = np.ascontiguousarray(
        np.concatenate([np.asarray(Wk, np.float32), np.asarray(Wv, np.float32)], axis=1)
    ).astype(BF)
    bqv = np.asarray(bq, np.float32).reshape(H, 1).copy()
    bkv = np.concatenate(
        [np.asarray(bk, np.float32), np.asarray(bv, np.float32)]
    ).reshape(2 * H, 1).copy()

    in_maps = []
    for core in range(N_CORES):
        b, half = divmod(core, 2)
        q0 = half * SQ
        hid_rot = np.roll(hidden_states[b], -q0, axis=0)
        in_maps.append(
            {
                "hidT": np.ascontiguousarray(hid_rot.T).astype(BF),
                "wq": wq,
                "wkv": wkv,
                "bq": bqv,
                "bkv": bkv,
            }
        )

    LAST_RESULT = run_bass_kernel_spmd(_NC, in_maps, core_ids=list(range(N_CORES)))
    out = np.empty((4, S, H), np.float32)
    for core in range(N_CORES):
        b, half = divmod(core, 2)
        q0 = half * SQ
        out[b, q0 : q0 + SQ] = LAST_RESULT.results[core]["out"]
    return out
